# revision 27
# baseline (speedup 1.0000x reference)
"""ConvAttention Trainium2 kernel (fp8 DoubleRow convs, exp(z) output).

Computes, for B=32 batches sharded 4-per-core across 8 NeuronCores:
  keys' = keys + style_emb^T                      (host, layout prep)
  k = conv1d_k1(relu(conv1d_k3(keys', kw1, kb1)), kw2, kb2)        [80, 400]
  q = conv1d_k1(relu(conv1d_k1(relu(conv1d_k3(queries, qw1, qb1)),
                               qw2, qb2)), qw3, qb3)               [80, 2000]
  z = SCALE * (|k|^2 - 2 q.k)            (the S*|q|^2 row-const cancels)
  e = exp(z)                             in [0.55, 1): exactly f16-safe
Device outputs e in f16; the host recovers (rsum = sum_t2 e):
  lsm  = log_softmax(attn_raw) = ln(e) - ln(rsum)
  logp = lsm + log(prior + eps) = ln(e * (prior+eps)) - ln(rsum)
  attn = softmax(where(mask, -inf, logp)) = e*(prior+eps)*keep / renorm.

Precision (rel-l2 vs f64 reference, measured on the real data):
  * k-path conv1 (512x3 -> 1024) and conv2 (1024 -> 80) run in fp8 e4m3
    with MatmulPerfMode.DoubleRow (2 contraction slices per instruction,
    0.5 cycles/row).  conv1 weights are pre-scaled x8 and y1 is stored
    x8 in fp8 (max 126 < 240); conv2 weights x16; the 1/128 is folded
    into the b0 assembly tensor_scalar.
  * everything else (q path, qk) is bf16.  attn ~5e-3, logp ~7e-4.

Schedule: q-paths of batches 0..2 run first as a software-pipelined
A/B/C chunk stream (conv1+relu / conv2+relu / conv3+add), hiding the
kw1/ks input DMA; then per batch: conv1 (8x 6 DoubleRow matmuls, relu
on DVE) with the previous batch's attention chunk-groups interleaved
(matmul -> paired Exp -> half-DMA, nothing else), conv2 + |k|^2 row
(deferred past the next block's first chunk so the gpsimd latency
hides).  Batch 3's q-path is software-pipelined with its own attention
groups at the end.

Perf notes (measured):
  * q-conv chunks are 400 cols: 500-free matmuls run ~2.7x slower per
    row than 400-free on this HW.
  * every dma_start costs ~600ns of issue time on its engine's queue,
    so weights are packed into single arenas (one DMA each).
  * each DMA ring moves only ~100 GB/s; output halves ride the gpsimd
    ring while inputs ride the SP ring.  scalar-engine DMA (the third
    hwdge ring) hard-crashes the device - do not use it.
"""

import os
import sys
import numpy as np

sys.path.insert(0, "/opt/trn_rl_repo")

B, T1, T2 = 32, 2000, 400
N_MEL, N_TEXT, N_ATT = 80, 512, 80
N_CORES = 8
BPC = B // N_CORES  # batches per core
SCALE = -0.0005
EPS = 1e-8
FOLD = -2.0 * SCALE     # 0.001, folded into b0 = FOLD*k
WS1 = 8.0               # conv1-k weight pre-scale (y1 stored x8, max ~126)
WS2 = 16.0              # conv2-k weight pre-scale

T1C = 125            # rows per attention chunk
NCHUNK = T1 // T1C   # 16
GRP = 4              # chunks per group (one output DMA per group)
NGRP = NCHUNK // GRP # 4
GW = GRP * T2        # 1600 free cols per group buffer

QT = 400             # t1 cols per q-conv chunk (500-free matmuls measure
NQT = T1 // QT       # 5   ~2.7x slower per row than 400-free on HW)

_PROGRAM_CACHE = {}


def build_program():
    import concourse.bass as bass
    import concourse.bacc as bacc
    import concourse.mybir as mybir
    from concourse import tile
    from concourse.hw_specs import get_activation_tables

    f32 = mybir.dt.float32
    f16 = mybir.dt.float16
    bf16 = mybir.dt.bfloat16
    u8 = mybir.dt.uint8
    f8 = mybir.dt.float8e4
    AF = mybir.ActivationFunctionType
    ALU = mybir.AluOpType
    DR = mybir.MatmulPerfMode.DoubleRow

    nc = bacc.Bacc("TRN2", target_bir_lowering=False, debug=False,
                   num_devices=N_CORES)

    # Pin Exp/Ln to one activation table set so the placement pass doesn't
    # thrash ACT_TABLE_LOADs between sets.
    tabs = get_activation_tables(nc.m.arch)
    for name, fns in tabs.items():
        if name != "natural_log_exp_and_others":
            fns.discard(AF.Exp)
            fns.discard(AF.Ln)

    # ---- I/O -------------------------------------------------------------
    # fp8 tensors cross the host boundary as uint8 and are bitcast at use.
    q_in_h = nc.dram_tensor("q_in", [BPC, N_MEL, T1 + 2], bf16, kind="ExternalInput").ap()
    ks8_h = nc.dram_tensor("ks8", [BPC, 128, 2, 2, T2 + 2], u8, kind="ExternalInput").ap()
    kw1f_h = nc.dram_tensor("kw1f", [2, 128, 3, 8, 2, 128], u8, kind="ExternalInput").ap()
    kw2f_h = nc.dram_tensor("kw2f", [128, 4, 2, N_ATT], u8, kind="ExternalInput").ap()
    qwp_h = nc.dram_tensor("qwp", [128, 720], bf16, kind="ExternalInput").ap()
    qbp_h = nc.dram_tensor("qbp", [128, 13], f32, kind="ExternalInput").ap()

    out_h = nc.dram_tensor("out_e", [BPC, NGRP, T1C, GRP, T2], f16,
                           kind="ExternalOutput").ap()

    with tile.TileContext(nc) as tc:
        from contextlib import ExitStack
        with ExitStack() as ctx:
            # PSUM pools first so slots are bank-aligned from offset 0:
            # ps_qk slots are 2 banks (4096B), ps_a slots 1 bank (2048B).
            ps_qk = ctx.enter_context(tc.tile_pool(name="ps_qk", bufs=2, space="PSUM"))
            ps_a = ctx.enter_context(tc.tile_pool(name="ps_a", bufs=4, space="PSUM"))

            const_pool = ctx.enter_context(tc.tile_pool(name="const", bufs=1))
            wpool = ctx.enter_context(tc.tile_pool(name="weights", bufs=1))
            kpath = ctx.enter_context(tc.tile_pool(name="kpath", bufs=2))
            y1pool = ctx.enter_context(tc.tile_pool(name="y1", bufs=2))
            qpath = ctx.enter_context(tc.tile_pool(name="qpath", bufs=1))
            qtmp = ctx.enter_context(tc.tile_pool(name="qtmp", bufs=3))
            chunkp = ctx.enter_context(tc.tile_pool(name="chunk", bufs=3))

            # ---- packed weights (one DMA per arena) ----------------------
            qwp = wpool.tile([128, 720], bf16, name="qwp")
            nc.sync.dma_start(out=qwp[:], in_=qwp_h[:, :])
            qbp = wpool.tile([128, 13], f32, name="qbp")
            nc.sync.dma_start(out=qbp[:], in_=qbp_h[:, :])

            # q weight views
            def qw1_ap(d, p0r, p1r):
                return qwp[0:N_MEL, 160 * d + p0r:160 * d + p1r]
            qw2_a = qwp[0:128, 480:560]
            qw2_b = qwp[0:32, 560:640]
            qw3_v = qwp[0:N_MEL, 640:720]
            qb1_a = qbp[0:128, 0:1]
            qb1_b = qbp[0:32, 1:2]
            qb2_v = qbp[0:N_MEL, 2:3]
            qb3_v = qbp[0:N_ATT, 3:4]
            kb2f_v = qbp[0:N_ATT, 4:5]
            kb1x8 = [qbp[0:128, 5 + j:6 + j] for j in range(8)]

            # q inputs for all batches (needed from the prologue on)
            q_in_sb = []
            for b in range(BPC):
                t = qpath.tile([N_MEL, T1 + 2], bf16, name=f"q_in_{b}")
                nc.sync.dma_start(out=t[:], in_=q_in_h[b, :, :])
                q_in_sb.append(t)

            # k inputs / weights
            ks_sb = {}

            def k_stage0(b):
                t = kpath.tile([128, 2, 2, T2 + 2], u8, name=f"ks_{b}",
                               tag="ks", bufs=2)
                nc.sync.dma_start(out=t[:], in_=ks8_h[b])
                ks_sb[b] = t

            k_stage0(0)
            kw1_sb = []
            for p in range(2):
                t = wpool.tile([128, 3, 8, 2, 128], u8, name=f"kw1_{p}")
                nc.sync.dma_start(out=t[:], in_=kw1f_h[p])
                kw1_sb.append(t)
            kw2_sb = wpool.tile([128, 4, 2, N_ATT], u8, name="kw2_sb")
            nc.sync.dma_start(out=kw2_sb[:], in_=kw2f_h[:])
            k_stage0(1)

            ones80 = const_pool.tile([N_ATT, 1], bf16, name="ones80")
            nc.vector.memset(ones80[:], 1.0)

            # q_fin per batch (persist; row 96 = ones for the |k|^2 row)
            q_fin_sb = []
            for b in range(BPC):
                t = qpath.tile([97, T1], bf16, name=f"q_fin_{b}")
                nc.vector.memset(t[64:97, :], 1.0)
                q_fin_sb.append(t)

            st = {}

            # ---------- q path (stage3): one 500-col chunk, 3 stages ------
            # A: conv1 matmuls + ACT relus; B: conv2 matmuls + DVE relu;
            # C: conv3 matmul + DVE bias-add into q_fin.  Emitted pipelined
            # (A(i), B(i-1), C(i-2)) so the PE never waits on ACT/DVE.
            qst = {}

            def qchunk_a(b, g):
                q_in = q_in_sb[b]
                t0 = g * QT
                y1qa = qtmp.tile([128, QT], bf16, name=f"y1qa_{b}_{g}", tag="y1qa", bufs=3)
                y1qb = qtmp.tile([32, QT], bf16, name=f"y1qb_{b}_{g}", tag="y1qb", bufs=3)
                for (p0r, p1r, bt, yt) in (
                    (0, 128, qb1_a, y1qa),
                    (128, 160, qb1_b, y1qb),
                ):
                    q1 = ps_a.tile([p1r - p0r, 512], f32, name=f"q1_{b}_{g}_{p0r}", tag="psa")
                    for d in range(3):
                        nc.tensor.matmul(q1[:, 0:QT], qw1_ap(d, p0r, p1r),
                                         q_in[:, d + t0:d + t0 + QT],
                                         start=(d == 0), stop=(d == 2))
                    nc.scalar.activation(yt[:], q1[:, 0:QT], AF.Relu, bias=bt)
                qst[(b, g)] = {"y1qa": y1qa, "y1qb": y1qb}

            def qchunk_b(b, g):
                s = qst[(b, g)]
                q2 = ps_a.tile([N_MEL, 512], f32, name=f"q2_{b}_{g}", tag="psa")
                nc.tensor.matmul(q2[:, 0:QT], qw2_a, s["y1qa"][:], start=True, stop=False)
                nc.tensor.matmul(q2[:, 0:QT], qw2_b, s["y1qb"][:], start=False, stop=True)
                q_mid = qtmp.tile([N_MEL, QT], bf16, name=f"q_mid_{b}_{g}", tag="q_mid", bufs=3)
                nc.vector.tensor_scalar(q_mid[:], q2[:, 0:QT], qb2_v, 0.0,
                                        ALU.add, ALU.max)
                s["q_mid"] = q_mid

            def qchunk_c(b, g):
                s = qst.pop((b, g))
                t0 = g * QT
                q3 = ps_a.tile([N_ATT, 512], f32, name=f"q3_{b}_{g}", tag="psa")
                nc.tensor.matmul(q3[:, 0:QT], qw3_v, s["q_mid"][:], start=True, stop=True)
                nc.vector.tensor_scalar_add(q_fin_sb[b][0:N_ATT, t0:t0 + QT],
                                            q3[:, 0:QT], qb3_v)

            # ---------- k path -------------------------------------------
            def conv1_chunk(b, j):
                """conv1 (k3, 512->1024) + relu(x8, on DVE) -> fp8."""
                s = st.setdefault(b, {})
                y1 = s.setdefault("y1", {})
                ks = ks_sb[b]
                c1 = ps_a.tile([128, 512], f32, name=f"c1_{b}_{j}", tag="psa")
                n = 0
                for p in range(2):
                    for d in range(3):
                        nc.tensor.matmul(
                            c1[:, 0:T2],
                            kw1_sb[p][:, d, j].bitcast(f8),
                            ks[:, p, :, d:d + T2].bitcast(f8),
                            start=(n == 0), stop=(n == 5),
                            perf_mode=DR,
                        )
                        n += 1
                if j % 2 == 0:
                    yp = y1pool.tile([128, 2, T2], f8, name=f"y1_{b}_{j // 2}",
                                     tag=f"y1_{j // 2}", bufs=2)
                    y1[j // 2] = yp
                # y1*8 = max(c1 + 8*kb1, 0)   (c1 is 8x because kw1 is x8)
                nc.vector.tensor_scalar(y1[j // 2][:, j % 2, :], c1[:, 0:T2],
                                        kb1x8[j], 0.0, ALU.add, ALU.max)

            def k_stage2(b):
                """conv2 (k1, 1024->80) + b0 rows + ksq; the |k|^2 matmul is
                deferred (k_stage2_fin) so the gpsimd latency hides under the
                next conv block's first chunk."""
                s = st[b]
                y1 = s["y1"]
                k_ps = ps_a.tile([N_ATT, 512], f32, name=f"k_ps_{b}", tag="psa")
                for p in range(4):
                    nc.tensor.matmul(k_ps[:, 0:T2], kw2_sb[:, p].bitcast(f8),
                                     y1[p][:], start=(p == 0), stop=(p == 3),
                                     perf_mode=DR)
                b0 = kpath.tile([97, T2], bf16, name=f"b0_{b}", tag="b0", bufs=2)
                nc.vector.memset(b0[64:97, :], 0.0)
                # b0 rows = FOLD*k = (FOLD/(WS1*WS2))*k_ps + FOLD*kb2
                nc.vector.tensor_scalar(b0[0:N_ATT, :], k_ps[:, 0:T2],
                                        FOLD / (WS1 * WS2), kb2f_v,
                                        ALU.mult, ALU.add)
                ksq = kpath.tile([N_ATT, T2], bf16, name=f"ksq_{b}", tag="ksq", bufs=2)
                nc.gpsimd.tensor_tensor(ksq[:], b0[0:N_ATT, :], b0[0:N_ATT, :],
                                        op=ALU.mult)
                s["ksq"] = ksq
                s["b0"] = b0

            def k_stage2_fin(b):
                """S*|k|^2 row: sum((FOLD*k)^2) / (4*SCALE) at b0 row 96."""
                s = st[b]
                k2_ps = ps_a.tile([1, 512], f32, name=f"k2_ps_{b}", tag="psa")
                nc.tensor.matmul(k2_ps[:, 0:T2], ones80[:], s["ksq"][:],
                                 start=True, stop=True)
                nc.vector.tensor_scalar_mul(s["b0"][96:97, :], k2_ps[:, 0:T2],
                                            1.0 / (4.0 * SCALE))

            # ---------- attention chunk group: matmul -> Exp -> DMA -------
            # out-DMAs alternate between the SP and ACT hardware DMA rings
            # (each ring is ~100 GB/s; all input DMAs ride the SP ring).
            def attn_group(b, g):
                b0 = st[b]["b0"]
                q_fin = q_fin_sb[b]
                o1 = chunkp.tile([T1C, GRP, T2], f16, name=f"o1_{b}_{g}",
                                 tag="o1", bufs=3)
                for half in range(2):
                    pp = ps_qk.tile([T1C, 2, 512], f32, name=f"pp_{b}_{g}_{half}",
                                    tag="psqk")
                    for r in range(2):
                        j = 2 * half + r
                        r0 = g * (GRP * T1C) + j * T1C
                        nc.tensor.matmul(pp[:, r, 0:T2], q_fin[:, r0:r0 + T1C],
                                         b0[:], start=True, stop=True)
                    nc.scalar.activation(o1[:, 2 * half:2 * half + 2, :],
                                         pp[:, :, 0:T2], AF.Exp)
                    # each half ships as soon as its Exp lands; the halves
                    # ride different hardware DMA rings (gpsimd / sync)
                    eng = nc.gpsimd if half == 0 else nc.sync
                    eng.dma_start(out=out_h[b, g, :, 2 * half:2 * half + 2, :],
                                  in_=o1[:, 2 * half:2 * half + 2, :])

            # ---------- emission ------------------------------------------
            # prologue: q paths for batches 0..2 as one pipelined stream of
            # 15 chunks (hides the kw1/ks DMA behind PE work).  B lags A by
            # 2 chunks and C lags B by 2 more so the PE never reaches a
            # matmul before its ACT/DVE producer has long finished (shorter
            # lags measure ~500-1000ns PE waits per chunk, which also hold
            # the PE p-state at 1.2 GHz).
            CH = [(b, g) for b in range(BPC - 1) for g in range(NQT)]
            for i in range(len(CH) + 4):
                if i < len(CH):
                    qchunk_a(*CH[i])
                if 0 <= i - 2 < len(CH):
                    qchunk_b(*CH[i - 2])
                if 0 <= i - 4 < len(CH):
                    qchunk_c(*CH[i - 4])

            # main: conv blocks; batch b-1's attention interleaves into
            # batch b's conv1; batch b+1's inputs prefetch during block b;
            # batch b-1's |k|^2 row finishes under block b's first chunk
            for b in range(BPC):
                for j in range(8):
                    conv1_chunk(b, j)
                    if j == 0:
                        if b >= 1:
                            k_stage2_fin(b - 1)
                        if 2 <= b + 1 < BPC:
                            k_stage0(b + 1)
                    if b >= 1 and j % 2 == 1:
                        attn_group(b - 1, (j - 1) // 2)
                k_stage2(b)

            # tail: batch 3's q path software-pipelined with its attention.
            # attn group g needs q_fin cols [500g, 500g+500), i.e. q chunks
            # through ceil(500(g+1)/QT)-1.
            bl = BPC - 1
            qchunk_a(bl, 0)
            k_stage2_fin(bl)
            qchunk_a(bl, 1)
            qchunk_a(bl, 2)
            qchunk_b(bl, 0)
            qchunk_a(bl, 3)
            qchunk_b(bl, 1)
            qchunk_c(bl, 0)
            qchunk_a(bl, 4)
            qchunk_b(bl, 2)
            qchunk_c(bl, 1)
            attn_group(bl, 0)
            qchunk_b(bl, 3)
            qchunk_c(bl, 2)
            attn_group(bl, 1)
            qchunk_b(bl, 4)
            qchunk_c(bl, 3)
            attn_group(bl, 2)
            qchunk_c(bl, 4)
            attn_group(bl, 3)

    nc.compile()
    return nc


def get_program():
    key = "prog_v3"
    if key not in _PROGRAM_CACHE:
        _PROGRAM_CACHE[key] = build_program()
    return _PROGRAM_CACHE[key]


def _np_dtypes():
    import ml_dtypes
    return np.dtype(ml_dtypes.bfloat16), np.dtype(ml_dtypes.float8_e4m3)


def make_in_maps(inputs):
    """Host-side prep: shard per core, transpose/fold/pack weights."""
    bf16, e4 = _np_dtypes()
    queries = np.asarray(inputs["queries"], np.float32)
    keys = np.asarray(inputs["keys"], np.float32)
    style = np.asarray(inputs["style_emb"], np.float32)

    qw1 = np.asarray(inputs["qw1"], np.float32)
    qb1 = np.asarray(inputs["qb1"], np.float32)
    qw2 = np.asarray(inputs["qw2"], np.float32)
    qb2 = np.asarray(inputs["qb2"], np.float32)
    qw3 = np.asarray(inputs["qw3"], np.float32)
    qb3 = np.asarray(inputs["qb3"], np.float32)
    kw1 = np.asarray(inputs["kw1"], np.float32)
    kb1 = np.asarray(inputs["kb1"], np.float32)
    kw2 = np.asarray(inputs["kw2"], np.float32)
    kb2 = np.asarray(inputs["kb2"], np.float32)

    # q input (bf16), zero-padded for the k3 conv
    q_p = np.zeros((B, N_MEL, T1 + 2), np.float32)
    q_p[:, :, 1:T1 + 1] = queries
    q_p = q_p.astype(bf16)

    # packed q weights [128, 720] bf16
    qwp = np.zeros((128, 720), np.float32)
    qw1t = qw1.transpose(2, 1, 0)                       # [3, 80, 160]
    for d in range(3):
        qwp[0:N_MEL, 160 * d:160 * (d + 1)] = qw1t[d]
    qw2t = qw2[:, :, 0].T                               # [160, 80]
    qwp[0:128, 480:560] = qw2t[0:128]
    qwp[0:32, 560:640] = qw2t[128:160]
    qwp[0:N_MEL, 640:720] = qw3[:, :, 0].T
    qwp = qwp.astype(bf16)

    # packed biases [128, 13] f32
    qbp = np.zeros((128, 13), np.float32)
    qbp[0:128, 0] = qb1[0:128]
    qbp[0:32, 1] = qb1[128:160]
    qbp[0:N_MEL, 2] = qb2
    qbp[0:N_ATT, 3] = qb3
    qbp[0:N_ATT, 4] = kb2 * FOLD
    kb1_2d = kb1.reshape(8, 128)
    for j in range(8):
        qbp[0:128, 5 + j] = kb1_2d[j] * WS1

    # k path (fp8): keys+style^T, padded: [B, k(128), p(2), r(2), 402]
    ksum = np.zeros((B, N_TEXT, T2 + 2), np.float32)
    ksum[:, :, 1:T2 + 1] = keys + style.transpose(0, 2, 1)
    ks8 = ksum.reshape(B, 2, 2, 128, T2 + 2).transpose(0, 3, 1, 2, 4)
    ks8 = np.ascontiguousarray(ks8).astype(e4).view(np.uint8)

    # kw1 x8: [p, k(128), d(3), j(8), r(2), m(128)]
    kw1t = kw1.transpose(2, 1, 0) * WS1                 # [3, 512, 1024]
    kw1f = kw1t.reshape(3, 2, 2, 128, 8, 128)           # [d,p,r,k,j,m]
    kw1f = kw1f.transpose(1, 3, 0, 4, 2, 5)              # [p,k,d,j,r,m]
    kw1f = np.ascontiguousarray(kw1f).astype(e4).view(np.uint8)

    # kw2 x16: [k(128), p(4), r(2), o(80)]
    kw2t = kw2[:, :, 0].T * WS2                          # [1024, 80]
    kw2f = kw2t.reshape(4, 2, 128, N_ATT).transpose(2, 0, 1, 3)
    kw2f = np.ascontiguousarray(kw2f).astype(e4).view(np.uint8)

    shared = dict(qwp=qwp, qbp=qbp, kw1f=kw1f, kw2f=kw2f)
    in_maps = []
    for c in range(N_CORES):
        sl = slice(c * BPC, (c + 1) * BPC)
        m = dict(shared)
        m["q_in"] = np.ascontiguousarray(q_p[sl])
        m["ks8"] = np.ascontiguousarray(ks8[sl])
        in_maps.append(m)
    return in_maps


def unpack_out(res_list, name="out_e"):
    """[BPC, NGRP, T1C, GRP, T2] f16 per core -> e [nb, 1, T1, T2] f32."""
    o = np.concatenate([np.asarray(r[name]) for r in res_list], axis=0)
    nb = o.shape[0]
    o = o.astype(np.float32).transpose(0, 1, 3, 2, 4).reshape(nb, 1, T1, T2)
    return np.ascontiguousarray(o)


def finish_host(e, attn_prior, mask):
    """From e = exp(z): logp = ln(e*(prior+eps)) - ln(rsum);
    attn = e*(prior+eps)*keep, renormalized."""
    nb = e.shape[0]
    prior = np.asarray(attn_prior, np.float32)[:nb]
    rsum = e.sum(axis=3, keepdims=True)                 # [nb,1,T1,1]
    ep = e * (prior[:, None] + EPS)
    logp = np.log(ep) - np.log(rsum)
    keep = ~np.asarray(mask[:nb, :, 0], bool)           # [nb, T2]
    u = ep * keep[:, None, None, :]
    u /= u.sum(axis=3, keepdims=True)
    return u.astype(np.float32), logp.astype(np.float32)


def run_cores(inputs, trace=False):
    from concourse.bass_utils import run_bass_kernel_spmd

    nc = get_program()
    in_maps = make_in_maps(inputs)
    res = run_bass_kernel_spmd(nc, in_maps, list(range(N_CORES)), trace=trace)
    e = unpack_out(res.results)
    attn, logp = finish_host(e, inputs["attn_prior"], np.asarray(inputs["mask"]))
    return attn, logp, res


def kernel(**inputs):
    attn, logp, _ = run_cores(inputs, trace=False)
    return attn, logp


# revision 28
# speedup vs baseline: 1.3005x; 1.3005x over previous
"""ConvAttention Trainium2 kernel (fp8 DoubleRow convs, exp(z) output).

Computes, for B=32 batches sharded 4-per-core across 8 NeuronCores:
  keys' = keys + style_emb^T                      (host, layout prep)
  k = conv1d_k1(relu(conv1d_k3(keys', kw1, kb1)), kw2, kb2)        [80, 400]
  q = conv1d_k1(relu(conv1d_k1(relu(conv1d_k3(queries, qw1, qb1)),
                               qw2, qb2)), qw3, qb3)               [80, 2000]
  z = SCALE * (|k|^2 - 2 q.k)            (the S*|q|^2 row-const cancels)
  e = exp(z)                             in [0.55, 1): exactly f16-safe
Device outputs e in f16; the host recovers (rsum = sum_t2 e):
  lsm  = log_softmax(attn_raw) = ln(e) - ln(rsum)
  logp = lsm + log(prior + eps) = ln(e * (prior+eps)) - ln(rsum)
  attn = softmax(where(mask, -inf, logp)) = e*(prior+eps)*keep / renorm.

Precision (rel-l2 vs f64 reference, measured on the real data):
  * k-path conv1 (512x3 -> 1024) and conv2 (1024 -> 80) run in fp8 e4m3
    with MatmulPerfMode.DoubleRow (2 contraction slices per instruction,
    0.5 cycles/row).  conv1 weights are pre-scaled x8 and y1 is stored
    x8 in fp8 (max 126 < 240); conv2 weights x16; the 1/128 is folded
    into the b0 assembly tensor_scalar.
  * everything else (q path, qk) is bf16.  attn ~5e-3, logp ~7e-4.

Schedule: q-paths of batches 0..2 run first as a software-pipelined
A/B/C chunk stream (conv1+relu / conv2+relu / conv3+add), hiding the
kw1/ks input DMA; then per batch: conv1 (8x 6 DoubleRow matmuls, relu
on DVE) with the previous batch's attention chunk-groups interleaved
(matmul -> paired Exp -> half-DMA, nothing else), conv2 + |k|^2 row
(deferred past the next block's first chunk so the gpsimd latency
hides).  Batch 3's q-path is software-pipelined with its own attention
groups at the end.

Perf notes (measured):
  * q-conv chunks are 400 cols: 500-free matmuls run ~2.7x slower per
    row than 400-free on this HW.
  * every dma_start costs ~600ns of issue time on its engine's queue,
    so weights are packed into single arenas (one DMA each).
  * each DMA ring moves only ~100 GB/s; output halves ride the gpsimd
    ring while inputs ride the SP ring.  scalar-engine DMA (the third
    hwdge ring) hard-crashes the device - do not use it.
"""

import os
import sys
import numpy as np

sys.path.insert(0, "/opt/trn_rl_repo")

B, T1, T2 = 32, 2000, 400
N_MEL, N_TEXT, N_ATT = 80, 512, 80
N_CORES = 8
BPC = B // N_CORES  # batches per core
SCALE = -0.0005
EPS = 1e-8
FOLD = -2.0 * SCALE     # 0.001, folded into b0 = FOLD*k
WS1 = 8.0               # conv1-k weight pre-scale (y1 stored x8, max ~126)
WS2 = 16.0              # conv2-k weight pre-scale

T1C = 125            # rows per attention chunk
NCHUNK = T1 // T1C   # 16
GRP = 4              # chunks per group (one output DMA per group)
NGRP = NCHUNK // GRP # 4
GW = GRP * T2        # 1600 free cols per group buffer

QT = 400             # t1 cols per q-conv chunk (500-free matmuls measure
NQT = T1 // QT       # 5   ~2.7x slower per row than 400-free on HW)

_PROGRAM_CACHE = {}


def build_program():
    import concourse.bass as bass
    import concourse.bacc as bacc
    import concourse.mybir as mybir
    from concourse import tile
    from concourse.hw_specs import get_activation_tables

    f32 = mybir.dt.float32
    f16 = mybir.dt.float16
    bf16 = mybir.dt.bfloat16
    u8 = mybir.dt.uint8
    f8 = mybir.dt.float8e4
    AF = mybir.ActivationFunctionType
    ALU = mybir.AluOpType
    DR = mybir.MatmulPerfMode.DoubleRow

    nc = bacc.Bacc("TRN2", target_bir_lowering=False, debug=False,
                   num_devices=N_CORES)

    # Pin Exp/Ln to one activation table set so the placement pass doesn't
    # thrash ACT_TABLE_LOADs between sets.
    tabs = get_activation_tables(nc.m.arch)
    for name, fns in tabs.items():
        if name != "natural_log_exp_and_others":
            fns.discard(AF.Exp)
            fns.discard(AF.Ln)

    # ---- I/O -------------------------------------------------------------
    # fp8 tensors cross the host boundary as uint8 and are bitcast at use.
    # q conv1 input with the 3 taps pre-shifted into the partition dim:
    # stack s = d*80+c (zero rows for s>=240), packed as [128, pair r, t];
    # one single-shot DoubleRow matmul per output group replaces the 3-tap
    # accumulation chain (accumulating matmuls measure ~2x slower per row).
    q8s_h = nc.dram_tensor("q8s", [BPC, 128, 2, T1], u8, kind="ExternalInput").ap()
    qw1s_h = nc.dram_tensor("qw1s", [128, 2, 160], u8, kind="ExternalInput").ap()
    ks8_h = nc.dram_tensor("ks8", [BPC, 128, 2, 2, T2 + 2], u8, kind="ExternalInput").ap()
    kw1f_h = nc.dram_tensor("kw1f", [2, 128, 3, 8, 2, 128], u8, kind="ExternalInput").ap()
    kw2f_h = nc.dram_tensor("kw2f", [128, 4, 2, N_ATT], u8, kind="ExternalInput").ap()
    qwp_h = nc.dram_tensor("qwp", [128, 720], bf16, kind="ExternalInput").ap()
    qbp_h = nc.dram_tensor("qbp", [128, 13], f32, kind="ExternalInput").ap()

    out_h = nc.dram_tensor("out_e", [BPC, NGRP, T1C, GRP, T2], f16,
                           kind="ExternalOutput").ap()

    with tile.TileContext(nc) as tc:
        from contextlib import ExitStack
        with ExitStack() as ctx:
            # PSUM pools first so slots are bank-aligned from offset 0:
            # ps_qk slots are 2 banks (4096B), ps_a slots 1 bank (2048B).
            ps_qk = ctx.enter_context(tc.tile_pool(name="ps_qk", bufs=2, space="PSUM"))
            ps_a = ctx.enter_context(tc.tile_pool(name="ps_a", bufs=4, space="PSUM"))

            const_pool = ctx.enter_context(tc.tile_pool(name="const", bufs=1))
            wpool = ctx.enter_context(tc.tile_pool(name="weights", bufs=1))
            kpath = ctx.enter_context(tc.tile_pool(name="kpath", bufs=2))
            y1pool = ctx.enter_context(tc.tile_pool(name="y1", bufs=2))
            qpath = ctx.enter_context(tc.tile_pool(name="qpath", bufs=1))
            qtmp = ctx.enter_context(tc.tile_pool(name="qtmp", bufs=3))
            chunkp = ctx.enter_context(tc.tile_pool(name="chunk", bufs=3))

            # ---- packed weights (one DMA per arena) ----------------------
            qwp = wpool.tile([128, 720], bf16, name="qwp")
            nc.sync.dma_start(out=qwp[:], in_=qwp_h[:, :])
            qbp = wpool.tile([128, 13], f32, name="qbp")
            nc.sync.dma_start(out=qbp[:], in_=qbp_h[:, :])

            # q conv1 stacked fp8 weights (x16), one DMA
            qw1s = wpool.tile([128, 2, 160], u8, name="qw1s")
            nc.sync.dma_start(out=qw1s[:], in_=qw1s_h[:])

            # q weight views
            qw2_a = qwp[0:128, 480:560]
            qw2_b = qwp[0:32, 560:640]
            qw3_v = qwp[0:N_MEL, 640:720]
            qb1_a = qbp[0:128, 0:1]
            qb1_b = qbp[0:32, 1:2]
            qb2_v = qbp[0:N_MEL, 2:3]
            qb3_v = qbp[0:N_ATT, 3:4]
            kb2f_v = qbp[0:N_ATT, 4:5]
            kb1x8 = [qbp[0:128, 5 + j:6 + j] for j in range(8)]

            # q inputs for all batches (needed from the prologue on)
            q_in_sb = []
            for b in range(BPC):
                t = qpath.tile([128, 2, T1], u8, name=f"q_in_{b}")
                nc.sync.dma_start(out=t[:], in_=q8s_h[b])
                q_in_sb.append(t)

            # k inputs / weights
            ks_sb = {}

            def k_stage0(b):
                t = kpath.tile([128, 2, 2, T2 + 2], u8, name=f"ks_{b}",
                               tag="ks", bufs=2)
                nc.sync.dma_start(out=t[:], in_=ks8_h[b])
                ks_sb[b] = t

            k_stage0(0)
            kw1_sb = []
            for p in range(2):
                t = wpool.tile([128, 3, 8, 2, 128], u8, name=f"kw1_{p}")
                nc.sync.dma_start(out=t[:], in_=kw1f_h[p])
                kw1_sb.append(t)
            kw2_sb = wpool.tile([128, 4, 2, N_ATT], u8, name="kw2_sb")
            nc.sync.dma_start(out=kw2_sb[:], in_=kw2f_h[:])
            k_stage0(1)

            ones80 = const_pool.tile([N_ATT, 1], bf16, name="ones80")
            nc.vector.memset(ones80[:], 1.0)

            # q_fin per batch (persist; row 96 = ones for the |k|^2 row)
            q_fin_sb = []
            for b in range(BPC):
                t = qpath.tile([97, T1], bf16, name=f"q_fin_{b}")
                nc.vector.memset(t[64:97, :], 1.0)
                q_fin_sb.append(t)

            st = {}

            # ---------- q path (stage3): one 500-col chunk, 3 stages ------
            # A: conv1 matmuls + ACT relus; B: conv2 matmuls + DVE relu;
            # C: conv3 matmul + DVE bias-add into q_fin.  Emitted pipelined
            # (A(i), B(i-1), C(i-2)) so the PE never waits on ACT/DVE.
            qst = {}

            def qchunk_a(b, g):
                q_in = q_in_sb[b]
                t0 = g * QT
                rhs = q_in[:, :, t0:t0 + QT].bitcast(f8)
                y1qa = qtmp.tile([128, QT], bf16, name=f"y1qa_{b}_{g}", tag="y1qa", bufs=3)
                y1qb = qtmp.tile([32, QT], bf16, name=f"y1qb_{b}_{g}", tag="y1qb", bufs=3)
                for (p0r, p1r, bt, yt) in (
                    (0, 128, qb1_a, y1qa),
                    (128, 160, qb1_b, y1qb),
                ):
                    q1 = ps_a.tile([p1r - p0r, 512], f32, name=f"q1_{b}_{g}_{p0r}", tag="psa")
                    nc.tensor.matmul(q1[:, 0:QT], qw1s[:, :, p0r:p1r].bitcast(f8),
                                     rhs, start=True, stop=True, perf_mode=DR)
                    # weights are x16 -> relu(psum/16 + qb1)
                    nc.scalar.activation(yt[:], q1[:, 0:QT], AF.Relu, bias=bt,
                                         scale=1.0 / 16.0)
                qst[(b, g)] = {"y1qa": y1qa, "y1qb": y1qb}

            def qchunk_b(b, g):
                s = qst[(b, g)]
                q2 = ps_a.tile([N_MEL, 512], f32, name=f"q2_{b}_{g}", tag="psa")
                nc.tensor.matmul(q2[:, 0:QT], qw2_a, s["y1qa"][:], start=True, stop=False)
                nc.tensor.matmul(q2[:, 0:QT], qw2_b, s["y1qb"][:], start=False, stop=True)
                q_mid = qtmp.tile([N_MEL, QT], bf16, name=f"q_mid_{b}_{g}", tag="q_mid", bufs=3)
                nc.vector.tensor_scalar(q_mid[:], q2[:, 0:QT], qb2_v, 0.0,
                                        ALU.add, ALU.max)
                s["q_mid"] = q_mid

            def qchunk_c(b, g):
                s = qst.pop((b, g))
                t0 = g * QT
                q3 = ps_a.tile([N_ATT, 512], f32, name=f"q3_{b}_{g}", tag="psa")
                nc.tensor.matmul(q3[:, 0:QT], qw3_v, s["q_mid"][:], start=True, stop=True)
                nc.vector.tensor_scalar_add(q_fin_sb[b][0:N_ATT, t0:t0 + QT],
                                            q3[:, 0:QT], qb3_v)

            # ---------- k path -------------------------------------------
            def conv1_chunk(b, j):
                """conv1 (k3, 512->1024) + relu(x8, on DVE) -> fp8."""
                s = st.setdefault(b, {})
                y1 = s.setdefault("y1", {})
                ks = ks_sb[b]
                c1 = ps_a.tile([128, 512], f32, name=f"c1_{b}_{j}", tag="psa")
                n = 0
                for p in range(2):
                    for d in range(3):
                        nc.tensor.matmul(
                            c1[:, 0:T2],
                            kw1_sb[p][:, d, j].bitcast(f8),
                            ks[:, p, :, d:d + T2].bitcast(f8),
                            start=(n == 0), stop=(n == 5),
                            perf_mode=DR,
                        )
                        n += 1
                if j % 2 == 0:
                    yp = y1pool.tile([128, 2, T2], f8, name=f"y1_{b}_{j // 2}",
                                     tag=f"y1_{j // 2}", bufs=2)
                    y1[j // 2] = yp
                # y1*8 = max(c1 + 8*kb1, 0)   (c1 is 8x because kw1 is x8)
                nc.vector.tensor_scalar(y1[j // 2][:, j % 2, :], c1[:, 0:T2],
                                        kb1x8[j], 0.0, ALU.add, ALU.max)

            def k_stage2(b):
                """conv2 (k1, 1024->80) + b0 rows + ksq; the |k|^2 matmul is
                deferred (k_stage2_fin) so the gpsimd latency hides under the
                next conv block's first chunk."""
                s = st[b]
                y1 = s["y1"]
                k_ps = ps_a.tile([N_ATT, 512], f32, name=f"k_ps_{b}", tag="psa")
                for p in range(4):
                    nc.tensor.matmul(k_ps[:, 0:T2], kw2_sb[:, p].bitcast(f8),
                                     y1[p][:], start=(p == 0), stop=(p == 3),
                                     perf_mode=DR)
                b0 = kpath.tile([97, T2], bf16, name=f"b0_{b}", tag="b0", bufs=2)
                nc.vector.memset(b0[64:97, :], 0.0)
                # b0 rows = FOLD*k = (FOLD/(WS1*WS2))*k_ps + FOLD*kb2
                nc.vector.tensor_scalar(b0[0:N_ATT, :], k_ps[:, 0:T2],
                                        FOLD / (WS1 * WS2), kb2f_v,
                                        ALU.mult, ALU.add)
                ksq = kpath.tile([N_ATT, T2], bf16, name=f"ksq_{b}", tag="ksq", bufs=2)
                nc.gpsimd.tensor_tensor(ksq[:], b0[0:N_ATT, :], b0[0:N_ATT, :],
                                        op=ALU.mult)
                s["ksq"] = ksq
                s["b0"] = b0

            def k_stage2_fin(b):
                """S*|k|^2 row: sum((FOLD*k)^2) / (4*SCALE) at b0 row 96."""
                s = st[b]
                k2_ps = ps_a.tile([1, 512], f32, name=f"k2_ps_{b}", tag="psa")
                nc.tensor.matmul(k2_ps[:, 0:T2], ones80[:], s["ksq"][:],
                                 start=True, stop=True)
                nc.vector.tensor_scalar_mul(s["b0"][96:97, :], k2_ps[:, 0:T2],
                                            1.0 / (4.0 * SCALE))

            # ---------- attention chunk group: matmul -> Exp -> DMA -------
            # out-DMAs alternate between the SP and ACT hardware DMA rings
            # (each ring is ~100 GB/s; all input DMAs ride the SP ring).
            def attn_group(b, g):
                b0 = st[b]["b0"]
                q_fin = q_fin_sb[b]
                o1 = chunkp.tile([T1C, GRP, T2], f16, name=f"o1_{b}_{g}",
                                 tag="o1", bufs=3)
                for half in range(2):
                    pp = ps_qk.tile([T1C, 2, 512], f32, name=f"pp_{b}_{g}_{half}",
                                    tag="psqk")
                    for r in range(2):
                        j = 2 * half + r
                        r0 = g * (GRP * T1C) + j * T1C
                        nc.tensor.matmul(pp[:, r, 0:T2], q_fin[:, r0:r0 + T1C],
                                         b0[:], start=True, stop=True)
                    nc.scalar.activation(o1[:, 2 * half:2 * half + 2, :],
                                         pp[:, :, 0:T2], AF.Exp)
                    # each half ships as soon as its Exp lands; the halves
                    # ride different hardware DMA rings (gpsimd / sync)
                    eng = nc.gpsimd if half == 0 else nc.sync
                    eng.dma_start(out=out_h[b, g, :, 2 * half:2 * half + 2, :],
                                  in_=o1[:, 2 * half:2 * half + 2, :])

            # ---------- emission ------------------------------------------
            # prologue: q paths for batches 0..2 as one pipelined stream of
            # 15 chunks (hides the kw1/ks DMA behind PE work).  B lags A by
            # 2 chunks and C lags B by 2 more so the PE never reaches a
            # matmul before its ACT/DVE producer has long finished (shorter
            # lags measure ~500-1000ns PE waits per chunk, which also hold
            # the PE p-state at 1.2 GHz).
            CH = [(b, g) for b in range(BPC - 1) for g in range(NQT)]
            for i in range(len(CH) + 4):
                if i < len(CH):
                    qchunk_a(*CH[i])
                if 0 <= i - 2 < len(CH):
                    qchunk_b(*CH[i - 2])
                if 0 <= i - 4 < len(CH):
                    qchunk_c(*CH[i - 4])

            # main: conv blocks; batch b-1's attention interleaves into
            # batch b's conv1; batch b+1's inputs prefetch during block b;
            # batch b-1's |k|^2 row finishes under block b's first chunk
            for b in range(BPC):
                for j in range(8):
                    conv1_chunk(b, j)
                    if j == 0:
                        if b >= 1:
                            k_stage2_fin(b - 1)
                        if 2 <= b + 1 < BPC:
                            k_stage0(b + 1)
                    if b >= 1 and j % 2 == 1:
                        attn_group(b - 1, (j - 1) // 2)
                k_stage2(b)

            # tail: batch 3's q path software-pipelined with its attention.
            # attn group g needs q_fin cols [500g, 500g+500), i.e. q chunks
            # through ceil(500(g+1)/QT)-1.
            bl = BPC - 1
            qchunk_a(bl, 0)
            k_stage2_fin(bl)
            qchunk_a(bl, 1)
            qchunk_a(bl, 2)
            qchunk_b(bl, 0)
            qchunk_a(bl, 3)
            qchunk_b(bl, 1)
            qchunk_c(bl, 0)
            qchunk_a(bl, 4)
            qchunk_b(bl, 2)
            qchunk_c(bl, 1)
            attn_group(bl, 0)
            qchunk_b(bl, 3)
            qchunk_c(bl, 2)
            attn_group(bl, 1)
            qchunk_b(bl, 4)
            qchunk_c(bl, 3)
            attn_group(bl, 2)
            qchunk_c(bl, 4)
            attn_group(bl, 3)

    nc.compile()
    return nc


def get_program():
    key = "prog_v3"
    if key not in _PROGRAM_CACHE:
        _PROGRAM_CACHE[key] = build_program()
    return _PROGRAM_CACHE[key]


def _np_dtypes():
    import ml_dtypes
    return np.dtype(ml_dtypes.bfloat16), np.dtype(ml_dtypes.float8_e4m3)


def make_in_maps(inputs):
    """Host-side prep: shard per core, transpose/fold/pack weights."""
    bf16, e4 = _np_dtypes()
    queries = np.asarray(inputs["queries"], np.float32)
    keys = np.asarray(inputs["keys"], np.float32)
    style = np.asarray(inputs["style_emb"], np.float32)

    qw1 = np.asarray(inputs["qw1"], np.float32)
    qb1 = np.asarray(inputs["qb1"], np.float32)
    qw2 = np.asarray(inputs["qw2"], np.float32)
    qb2 = np.asarray(inputs["qb2"], np.float32)
    qw3 = np.asarray(inputs["qw3"], np.float32)
    qb3 = np.asarray(inputs["qb3"], np.float32)
    kw1 = np.asarray(inputs["kw1"], np.float32)
    kb1 = np.asarray(inputs["kb1"], np.float32)
    kw2 = np.asarray(inputs["kw2"], np.float32)
    kb2 = np.asarray(inputs["kb2"], np.float32)

    # q conv1 input, taps pre-shifted into the partition dim (fp8):
    # stack[s=d*80+c, t] = x_pad[c, t+d], zero rows 240..255, as [128,2,T1]
    x_pad = np.zeros((B, N_MEL, T1 + 2), np.float32)
    x_pad[:, :, 1:T1 + 1] = queries
    stack = np.concatenate([x_pad[:, :, d:d + T1] for d in range(3)] +
                           [np.zeros((B, 16, T1), np.float32)], axis=1)
    q8s = stack.reshape(B, 2, 128, T1).transpose(0, 2, 1, 3)
    q8s = np.ascontiguousarray(q8s).astype(e4).view(np.uint8)

    # stacked q conv1 weights x16: [k, r, m] = qw1[m, c, d], s = 128r+k
    qw1t16 = qw1.transpose(2, 1, 0).reshape(240, 160) * 16.0
    qw1s = np.concatenate([qw1t16, np.zeros((16, 160), np.float32)], axis=0)
    qw1s = np.ascontiguousarray(qw1s.reshape(2, 128, 160).transpose(1, 0, 2)
                                ).astype(e4).view(np.uint8)

    # packed q weights [128, 720] bf16
    qwp = np.zeros((128, 720), np.float32)
    qw1t = qw1.transpose(2, 1, 0)                       # [3, 80, 160]
    for d in range(3):
        qwp[0:N_MEL, 160 * d:160 * (d + 1)] = qw1t[d]
    qw2t = qw2[:, :, 0].T                               # [160, 80]
    qwp[0:128, 480:560] = qw2t[0:128]
    qwp[0:32, 560:640] = qw2t[128:160]
    qwp[0:N_MEL, 640:720] = qw3[:, :, 0].T
    qwp = qwp.astype(bf16)

    # packed biases [128, 13] f32
    qbp = np.zeros((128, 13), np.float32)
    qbp[0:128, 0] = qb1[0:128]
    qbp[0:32, 1] = qb1[128:160]
    qbp[0:N_MEL, 2] = qb2
    qbp[0:N_ATT, 3] = qb3
    qbp[0:N_ATT, 4] = kb2 * FOLD
    kb1_2d = kb1.reshape(8, 128)
    for j in range(8):
        qbp[0:128, 5 + j] = kb1_2d[j] * WS1

    # k path (fp8): keys+style^T, padded: [B, k(128), p(2), r(2), 402]
    ksum = np.zeros((B, N_TEXT, T2 + 2), np.float32)
    ksum[:, :, 1:T2 + 1] = keys + style.transpose(0, 2, 1)
    ks8 = ksum.reshape(B, 2, 2, 128, T2 + 2).transpose(0, 3, 1, 2, 4)
    ks8 = np.ascontiguousarray(ks8).astype(e4).view(np.uint8)

    # kw1 x8: [p, k(128), d(3), j(8), r(2), m(128)]
    kw1t = kw1.transpose(2, 1, 0) * WS1                 # [3, 512, 1024]
    kw1f = kw1t.reshape(3, 2, 2, 128, 8, 128)           # [d,p,r,k,j,m]
    kw1f = kw1f.transpose(1, 3, 0, 4, 2, 5)              # [p,k,d,j,r,m]
    kw1f = np.ascontiguousarray(kw1f).astype(e4).view(np.uint8)

    # kw2 x16: [k(128), p(4), r(2), o(80)]
    kw2t = kw2[:, :, 0].T * WS2                          # [1024, 80]
    kw2f = kw2t.reshape(4, 2, 128, N_ATT).transpose(2, 0, 1, 3)
    kw2f = np.ascontiguousarray(kw2f).astype(e4).view(np.uint8)

    shared = dict(qwp=qwp, qbp=qbp, kw1f=kw1f, kw2f=kw2f, qw1s=qw1s)
    in_maps = []
    for c in range(N_CORES):
        sl = slice(c * BPC, (c + 1) * BPC)
        m = dict(shared)
        m["q8s"] = np.ascontiguousarray(q8s[sl])
        m["ks8"] = np.ascontiguousarray(ks8[sl])
        in_maps.append(m)
    return in_maps


def unpack_out(res_list, name="out_e"):
    """[BPC, NGRP, T1C, GRP, T2] f16 per core -> e [nb, 1, T1, T2] f32."""
    o = np.concatenate([np.asarray(r[name]) for r in res_list], axis=0)
    nb = o.shape[0]
    o = o.astype(np.float32).transpose(0, 1, 3, 2, 4).reshape(nb, 1, T1, T2)
    return np.ascontiguousarray(o)


def finish_host(e, attn_prior, mask):
    """From e = exp(z): logp = ln(e*(prior+eps)) - ln(rsum);
    attn = e*(prior+eps)*keep, renormalized."""
    nb = e.shape[0]
    prior = np.asarray(attn_prior, np.float32)[:nb]
    rsum = e.sum(axis=3, keepdims=True)                 # [nb,1,T1,1]
    ep = e * (prior[:, None] + EPS)
    logp = np.log(ep) - np.log(rsum)
    keep = ~np.asarray(mask[:nb, :, 0], bool)           # [nb, T2]
    u = ep * keep[:, None, None, :]
    u /= u.sum(axis=3, keepdims=True)
    return u.astype(np.float32), logp.astype(np.float32)


def run_cores(inputs, trace=False):
    from concourse.bass_utils import run_bass_kernel_spmd

    nc = get_program()
    in_maps = make_in_maps(inputs)
    res = run_bass_kernel_spmd(nc, in_maps, list(range(N_CORES)), trace=trace)
    e = unpack_out(res.results)
    attn, logp = finish_host(e, inputs["attn_prior"], np.asarray(inputs["mask"]))
    return attn, logp, res


def kernel(**inputs):
    attn, logp, _ = run_cores(inputs, trace=False)
    return attn, logp


# revision 29
# speedup vs baseline: 1.3016x; 1.0008x over previous
"""ConvAttention Trainium2 kernel (fp8 DoubleRow convs, exp(z) output).

Computes, for B=32 batches sharded 4-per-core across 8 NeuronCores:
  keys' = keys + style_emb^T                      (host, layout prep)
  k = conv1d_k1(relu(conv1d_k3(keys', kw1, kb1)), kw2, kb2)        [80, 400]
  q = conv1d_k1(relu(conv1d_k1(relu(conv1d_k3(queries, qw1, qb1)),
                               qw2, qb2)), qw3, qb3)               [80, 2000]
  z = SCALE * (|k|^2 - 2 q.k)            (the S*|q|^2 row-const cancels)
  e = exp(z)                             in [0.55, 1): exactly f16-safe
Device outputs e in f16; the host recovers (rsum = sum_t2 e):
  lsm  = log_softmax(attn_raw) = ln(e) - ln(rsum)
  logp = lsm + log(prior + eps) = ln(e * (prior+eps)) - ln(rsum)
  attn = softmax(where(mask, -inf, logp)) = e*(prior+eps)*keep / renorm.

Precision (rel-l2 vs f64 reference, measured on the real data):
  * k-path conv1 (512x3 -> 1024) and conv2 (1024 -> 80) run in fp8 e4m3
    with MatmulPerfMode.DoubleRow (2 contraction slices per instruction,
    0.5 cycles/row).  conv1 weights are pre-scaled x8 and y1 is stored
    x8 in fp8 (max 126 < 240); conv2 weights x16; the 1/128 is folded
    into the b0 assembly tensor_scalar.
  * everything else (q path, qk) is bf16.  attn ~5e-3, logp ~7e-4.

Schedule: q-paths of batches 0..2 run first as a software-pipelined
A/B/C chunk stream (conv1+relu / conv2+relu / conv3+add), hiding the
kw1/ks input DMA; then per batch: conv1 (8x 6 DoubleRow matmuls, relu
on DVE) with the previous batch's attention chunk-groups interleaved
(matmul -> paired Exp -> half-DMA, nothing else), conv2 + |k|^2 row
(deferred past the next block's first chunk so the gpsimd latency
hides).  Batch 3's q-path is software-pipelined with its own attention
groups at the end.

Perf notes (measured):
  * q-conv chunks are 400 cols: 500-free matmuls run ~2.7x slower per
    row than 400-free on this HW.
  * every dma_start costs ~600ns of issue time on its engine's queue,
    so weights are packed into single arenas (one DMA each).
  * each DMA ring moves only ~100 GB/s; output halves ride the gpsimd
    ring while inputs ride the SP ring.  scalar-engine DMA (the third
    hwdge ring) hard-crashes the device - do not use it.
"""

import os
import sys
import numpy as np

sys.path.insert(0, "/opt/trn_rl_repo")

B, T1, T2 = 32, 2000, 400
N_MEL, N_TEXT, N_ATT = 80, 512, 80
N_CORES = 8
BPC = B // N_CORES  # batches per core
SCALE = -0.0005
EPS = 1e-8
FOLD = -2.0 * SCALE     # 0.001, folded into b0 = FOLD*k
WS1 = 8.0               # conv1-k weight pre-scale (y1 stored x8, max ~126)
WS2 = 16.0              # conv2-k weight pre-scale

T1C = 125            # rows per attention chunk
NCHUNK = T1 // T1C   # 16
GRP = 4              # chunks per group (one output DMA per group)
NGRP = NCHUNK // GRP # 4
GW = GRP * T2        # 1600 free cols per group buffer

QT = 400             # t1 cols per q-conv chunk (500-free matmuls measure
NQT = T1 // QT       # 5   ~2.7x slower per row than 400-free on HW)

_PROGRAM_CACHE = {}


def build_program():
    import concourse.bass as bass
    import concourse.bacc as bacc
    import concourse.mybir as mybir
    from concourse import tile
    from concourse.hw_specs import get_activation_tables

    f32 = mybir.dt.float32
    f16 = mybir.dt.float16
    bf16 = mybir.dt.bfloat16
    u8 = mybir.dt.uint8
    f8 = mybir.dt.float8e4
    AF = mybir.ActivationFunctionType
    ALU = mybir.AluOpType
    DR = mybir.MatmulPerfMode.DoubleRow

    nc = bacc.Bacc("TRN2", target_bir_lowering=False, debug=False,
                   num_devices=N_CORES)

    # Pin Exp/Ln to one activation table set so the placement pass doesn't
    # thrash ACT_TABLE_LOADs between sets.
    tabs = get_activation_tables(nc.m.arch)
    for name, fns in tabs.items():
        if name != "natural_log_exp_and_others":
            fns.discard(AF.Exp)
            fns.discard(AF.Ln)

    # ---- I/O -------------------------------------------------------------
    # fp8 tensors cross the host boundary as uint8 and are bitcast at use.
    # q conv1 input with the 3 taps pre-shifted into the partition dim:
    # stack s = d*80+c (zero rows for s>=240), packed as [128, pair r, t];
    # one single-shot DoubleRow matmul per output group replaces the 3-tap
    # accumulation chain (accumulating matmuls measure ~2x slower per row).
    q8s_h = nc.dram_tensor("q8s", [BPC, 128, 2, T1], u8, kind="ExternalInput").ap()
    qw1s_h = nc.dram_tensor("qw1s", [128, 2, 256], u8, kind="ExternalInput").ap()
    qw2s_h = nc.dram_tensor("qw2s", [128, 2, N_ATT], u8, kind="ExternalInput").ap()
    ks8_h = nc.dram_tensor("ks8", [BPC, 128, 2, 2, T2 + 2], u8, kind="ExternalInput").ap()
    kw1f_h = nc.dram_tensor("kw1f", [2, 128, 3, 8, 2, 128], u8, kind="ExternalInput").ap()
    kw2f_h = nc.dram_tensor("kw2f", [128, 4, 2, N_ATT], u8, kind="ExternalInput").ap()
    qwp_h = nc.dram_tensor("qwp", [128, 720], bf16, kind="ExternalInput").ap()
    qbp_h = nc.dram_tensor("qbp", [128, 13], f32, kind="ExternalInput").ap()

    out_h = nc.dram_tensor("out_e", [BPC, NGRP, T1C, GRP, T2], f16,
                           kind="ExternalOutput").ap()

    with tile.TileContext(nc) as tc:
        from contextlib import ExitStack
        with ExitStack() as ctx:
            # PSUM pools first so slots are bank-aligned from offset 0:
            # ps_qk slots are 2 banks (4096B), ps_a slots 1 bank (2048B).
            ps_qk = ctx.enter_context(tc.tile_pool(name="ps_qk", bufs=2, space="PSUM"))
            ps_a = ctx.enter_context(tc.tile_pool(name="ps_a", bufs=4, space="PSUM"))

            const_pool = ctx.enter_context(tc.tile_pool(name="const", bufs=1))
            wpool = ctx.enter_context(tc.tile_pool(name="weights", bufs=1))
            kpath = ctx.enter_context(tc.tile_pool(name="kpath", bufs=2))
            y1pool = ctx.enter_context(tc.tile_pool(name="y1", bufs=2))
            qpath = ctx.enter_context(tc.tile_pool(name="qpath", bufs=1))
            qtmp = ctx.enter_context(tc.tile_pool(name="qtmp", bufs=3))
            chunkp = ctx.enter_context(tc.tile_pool(name="chunk", bufs=3))

            # ---- packed weights (one DMA per arena) ----------------------
            qwp = wpool.tile([128, 720], bf16, name="qwp")
            nc.sync.dma_start(out=qwp[:], in_=qwp_h[:, :])
            qbp = wpool.tile([128, 13], f32, name="qbp")
            nc.sync.dma_start(out=qbp[:], in_=qbp_h[:, :])

            # q conv1/conv2 stacked fp8 weights, one DMA each
            qw1s = wpool.tile([128, 2, 256], u8, name="qw1s")
            nc.sync.dma_start(out=qw1s[:], in_=qw1s_h[:])
            qw2s = wpool.tile([128, 2, N_ATT], u8, name="qw2s")
            nc.sync.dma_start(out=qw2s[:], in_=qw2s_h[:])

            # q weight views
            qw2_a = qwp[0:128, 480:560]
            qw2_b = qwp[0:32, 560:640]
            qw3_v = qwp[0:N_MEL, 640:720]
            qb1_a = qbp[0:128, 0:1]
            qb1_b = qbp[0:32, 1:2]
            qb2_v = qbp[0:N_MEL, 2:3]
            qb3_v = qbp[0:N_ATT, 3:4]
            kb2f_v = qbp[0:N_ATT, 4:5]
            kb1x8 = [qbp[0:128, 5 + j:6 + j] for j in range(8)]

            # q inputs for all batches (needed from the prologue on)
            q_in_sb = []
            for b in range(BPC):
                t = qpath.tile([128, 2, T1], u8, name=f"q_in_{b}")
                nc.sync.dma_start(out=t[:], in_=q8s_h[b])
                q_in_sb.append(t)

            # k inputs / weights
            ks_sb = {}

            def k_stage0(b):
                t = kpath.tile([128, 2, 2, T2 + 2], u8, name=f"ks_{b}",
                               tag="ks", bufs=2)
                nc.sync.dma_start(out=t[:], in_=ks8_h[b])
                ks_sb[b] = t

            k_stage0(0)
            kw1_sb = []
            for p in range(2):
                t = wpool.tile([128, 3, 8, 2, 128], u8, name=f"kw1_{p}")
                nc.sync.dma_start(out=t[:], in_=kw1f_h[p])
                kw1_sb.append(t)
            kw2_sb = wpool.tile([128, 4, 2, N_ATT], u8, name="kw2_sb")
            nc.sync.dma_start(out=kw2_sb[:], in_=kw2f_h[:])
            k_stage0(1)

            ones80 = const_pool.tile([N_ATT, 1], bf16, name="ones80")
            nc.vector.memset(ones80[:], 1.0)

            # q_fin per batch (persist; row 96 = ones for the |k|^2 row)
            q_fin_sb = []
            for b in range(BPC):
                t = qpath.tile([97, T1], bf16, name=f"q_fin_{b}")
                nc.vector.memset(t[64:97, :], 1.0)
                q_fin_sb.append(t)

            st = {}

            # ---------- q path (stage3): one 500-col chunk, 3 stages ------
            # A: conv1 matmuls + ACT relus; B: conv2 matmuls + DVE relu;
            # C: conv3 matmul + DVE bias-add into q_fin.  Emitted pipelined
            # (A(i), B(i-1), C(i-2)) so the PE never waits on ACT/DVE.
            qst = {}

            def qchunk_a(b, g):
                q_in = q_in_sb[b]
                t0 = g * QT
                rhs = q_in[:, :, t0:t0 + QT].bitcast(f8)
                # both relu outputs land in one fp8 pair tile; the group-b
                # weights (and biases) are zero beyond row 32, so its psum
                # rows 32..127 are exact zeros -> relu writes zeros there
                y1q = qtmp.tile([128, 2, QT], f8, name=f"y1q_{b}_{g}", tag="y1q", bufs=3)
                for (r, m0, bt) in ((0, 0, qb1_a), (1, 128, qbp[0:128, 1:2])):
                    q1 = ps_a.tile([128, 512], f32, name=f"q1_{b}_{g}_{r}", tag="psa")
                    nc.tensor.matmul(q1[:, 0:QT], qw1s[:, :, m0:m0 + 128].bitcast(f8),
                                     rhs, start=True, stop=True, perf_mode=DR)
                    # weights are x16 -> relu(psum/16 + qb1)
                    nc.scalar.activation(y1q[:, r, :], q1[:, 0:QT], AF.Relu,
                                         bias=bt, scale=1.0 / 16.0)
                qst[(b, g)] = {"y1q": y1q}

            def qchunk_b(b, g):
                s = qst[(b, g)]
                q2 = ps_a.tile([N_MEL, 512], f32, name=f"q2_{b}_{g}", tag="psa")
                nc.tensor.matmul(q2[:, 0:QT], qw2s[:].bitcast(f8), s["y1q"][:],
                                 start=True, stop=True, perf_mode=DR)
                q_mid = qtmp.tile([N_MEL, QT], bf16, name=f"q_mid_{b}_{g}", tag="q_mid", bufs=3)
                nc.vector.tensor_scalar(q_mid[:], q2[:, 0:QT], qb2_v, 0.0,
                                        ALU.add, ALU.max)
                s["q_mid"] = q_mid

            def qchunk_c(b, g):
                s = qst.pop((b, g))
                t0 = g * QT
                q3 = ps_a.tile([N_ATT, 512], f32, name=f"q3_{b}_{g}", tag="psa")
                nc.tensor.matmul(q3[:, 0:QT], qw3_v, s["q_mid"][:], start=True, stop=True)
                nc.vector.tensor_scalar_add(q_fin_sb[b][0:N_ATT, t0:t0 + QT],
                                            q3[:, 0:QT], qb3_v)

            # ---------- k path -------------------------------------------
            def conv1_chunk(b, j):
                """conv1 (k3, 512->1024) + relu(x8, on DVE) -> fp8."""
                s = st.setdefault(b, {})
                y1 = s.setdefault("y1", {})
                ks = ks_sb[b]
                c1 = ps_a.tile([128, 512], f32, name=f"c1_{b}_{j}", tag="psa")
                n = 0
                for p in range(2):
                    for d in range(3):
                        nc.tensor.matmul(
                            c1[:, 0:T2],
                            kw1_sb[p][:, d, j].bitcast(f8),
                            ks[:, p, :, d:d + T2].bitcast(f8),
                            start=(n == 0), stop=(n == 5),
                            perf_mode=DR,
                        )
                        n += 1
                if j % 2 == 0:
                    yp = y1pool.tile([128, 2, T2], f8, name=f"y1_{b}_{j // 2}",
                                     tag=f"y1_{j // 2}", bufs=2)
                    y1[j // 2] = yp
                # y1*8 = max(c1 + 8*kb1, 0)   (c1 is 8x because kw1 is x8)
                nc.vector.tensor_scalar(y1[j // 2][:, j % 2, :], c1[:, 0:T2],
                                        kb1x8[j], 0.0, ALU.add, ALU.max)

            def k_stage2(b):
                """conv2 (k1, 1024->80) + b0 rows + ksq; the |k|^2 matmul is
                deferred (k_stage2_fin) so the gpsimd latency hides under the
                next conv block's first chunk."""
                s = st[b]
                y1 = s["y1"]
                k_ps = ps_a.tile([N_ATT, 512], f32, name=f"k_ps_{b}", tag="psa")
                for p in range(4):
                    nc.tensor.matmul(k_ps[:, 0:T2], kw2_sb[:, p].bitcast(f8),
                                     y1[p][:], start=(p == 0), stop=(p == 3),
                                     perf_mode=DR)
                b0 = kpath.tile([97, T2], bf16, name=f"b0_{b}", tag="b0", bufs=2)
                nc.vector.memset(b0[64:97, :], 0.0)
                # b0 rows = FOLD*k = (FOLD/(WS1*WS2))*k_ps + FOLD*kb2
                nc.vector.tensor_scalar(b0[0:N_ATT, :], k_ps[:, 0:T2],
                                        FOLD / (WS1 * WS2), kb2f_v,
                                        ALU.mult, ALU.add)
                ksq = kpath.tile([N_ATT, T2], bf16, name=f"ksq_{b}", tag="ksq", bufs=2)
                nc.gpsimd.tensor_tensor(ksq[:], b0[0:N_ATT, :], b0[0:N_ATT, :],
                                        op=ALU.mult)
                s["ksq"] = ksq
                s["b0"] = b0

            def k_stage2_fin(b):
                """S*|k|^2 row: sum((FOLD*k)^2) / (4*SCALE) at b0 row 96."""
                s = st[b]
                k2_ps = ps_a.tile([1, 512], f32, name=f"k2_ps_{b}", tag="psa")
                nc.tensor.matmul(k2_ps[:, 0:T2], ones80[:], s["ksq"][:],
                                 start=True, stop=True)
                nc.vector.tensor_scalar_mul(s["b0"][96:97, :], k2_ps[:, 0:T2],
                                            1.0 / (4.0 * SCALE))

            # ---------- attention chunk group: matmul -> Exp -> DMA -------
            # out-DMAs alternate between the SP and ACT hardware DMA rings
            # (each ring is ~100 GB/s; all input DMAs ride the SP ring).
            def attn_group(b, g):
                b0 = st[b]["b0"]
                q_fin = q_fin_sb[b]
                o1 = chunkp.tile([T1C, GRP, T2], f16, name=f"o1_{b}_{g}",
                                 tag="o1", bufs=3)
                for half in range(2):
                    pp = ps_qk.tile([T1C, 2, 512], f32, name=f"pp_{b}_{g}_{half}",
                                    tag="psqk")
                    for r in range(2):
                        j = 2 * half + r
                        r0 = g * (GRP * T1C) + j * T1C
                        nc.tensor.matmul(pp[:, r, 0:T2], q_fin[:, r0:r0 + T1C],
                                         b0[:], start=True, stop=True)
                    nc.scalar.activation(o1[:, 2 * half:2 * half + 2, :],
                                         pp[:, :, 0:T2], AF.Exp)
                    # each half ships as soon as its Exp lands; the halves
                    # ride different hardware DMA rings (gpsimd / sync)
                    eng = nc.gpsimd if half == 0 else nc.sync
                    eng.dma_start(out=out_h[b, g, :, 2 * half:2 * half + 2, :],
                                  in_=o1[:, 2 * half:2 * half + 2, :])

            # ---------- emission ------------------------------------------
            # prologue: q paths for batches 0..2 as one pipelined stream of
            # 15 chunks (hides the kw1/ks DMA behind PE work).  B lags A by
            # 2 chunks and C lags B by 2 more so the PE never reaches a
            # matmul before its ACT/DVE producer has long finished (shorter
            # lags measure ~500-1000ns PE waits per chunk, which also hold
            # the PE p-state at 1.2 GHz).
            CH = [(b, g) for b in range(BPC - 1) for g in range(NQT)]
            for i in range(len(CH) + 4):
                if i < len(CH):
                    qchunk_a(*CH[i])
                if 0 <= i - 2 < len(CH):
                    qchunk_b(*CH[i - 2])
                if 0 <= i - 4 < len(CH):
                    qchunk_c(*CH[i - 4])

            # main: conv blocks; batch b-1's attention interleaves into
            # batch b's conv1; batch b+1's inputs prefetch during block b;
            # batch b-1's |k|^2 row finishes under block b's first chunk
            for b in range(BPC):
                for j in range(8):
                    conv1_chunk(b, j)
                    if j == 0:
                        if b >= 1:
                            k_stage2_fin(b - 1)
                        if 2 <= b + 1 < BPC:
                            k_stage0(b + 1)
                    if b >= 1 and j % 2 == 1:
                        attn_group(b - 1, (j - 1) // 2)
                k_stage2(b)

            # tail: batch 3's q path software-pipelined with its attention.
            # attn group g needs q_fin cols [500g, 500g+500), i.e. q chunks
            # through ceil(500(g+1)/QT)-1.
            bl = BPC - 1
            qchunk_a(bl, 0)
            k_stage2_fin(bl)
            qchunk_a(bl, 1)
            qchunk_a(bl, 2)
            qchunk_b(bl, 0)
            qchunk_a(bl, 3)
            qchunk_b(bl, 1)
            qchunk_c(bl, 0)
            qchunk_a(bl, 4)
            qchunk_b(bl, 2)
            qchunk_c(bl, 1)
            attn_group(bl, 0)
            qchunk_b(bl, 3)
            qchunk_c(bl, 2)
            attn_group(bl, 1)
            qchunk_b(bl, 4)
            qchunk_c(bl, 3)
            attn_group(bl, 2)
            qchunk_c(bl, 4)
            attn_group(bl, 3)

    nc.compile()
    return nc


def get_program():
    key = "prog_v3"
    if key not in _PROGRAM_CACHE:
        _PROGRAM_CACHE[key] = build_program()
    return _PROGRAM_CACHE[key]


def _np_dtypes():
    import ml_dtypes
    return np.dtype(ml_dtypes.bfloat16), np.dtype(ml_dtypes.float8_e4m3)


def make_in_maps(inputs):
    """Host-side prep: shard per core, transpose/fold/pack weights."""
    bf16, e4 = _np_dtypes()
    queries = np.asarray(inputs["queries"], np.float32)
    keys = np.asarray(inputs["keys"], np.float32)
    style = np.asarray(inputs["style_emb"], np.float32)

    qw1 = np.asarray(inputs["qw1"], np.float32)
    qb1 = np.asarray(inputs["qb1"], np.float32)
    qw2 = np.asarray(inputs["qw2"], np.float32)
    qb2 = np.asarray(inputs["qb2"], np.float32)
    qw3 = np.asarray(inputs["qw3"], np.float32)
    qb3 = np.asarray(inputs["qb3"], np.float32)
    kw1 = np.asarray(inputs["kw1"], np.float32)
    kb1 = np.asarray(inputs["kb1"], np.float32)
    kw2 = np.asarray(inputs["kw2"], np.float32)
    kb2 = np.asarray(inputs["kb2"], np.float32)

    # q conv1 input, taps pre-shifted into the partition dim (fp8):
    # stack[s=d*80+c, t] = x_pad[c, t+d], zero rows 240..255, as [128,2,T1]
    x_pad = np.zeros((B, N_MEL, T1 + 2), np.float32)
    x_pad[:, :, 1:T1 + 1] = queries
    stack = np.concatenate([x_pad[:, :, d:d + T1] for d in range(3)] +
                           [np.zeros((B, 16, T1), np.float32)], axis=1)
    q8s = stack.reshape(B, 2, 128, T1).transpose(0, 2, 1, 3)
    q8s = np.ascontiguousarray(q8s).astype(e4).view(np.uint8)

    # stacked q conv1 weights x16: [k, r, m] = qw1[m, c, d], s = 128r+k;
    # out cols 0:128 = channels 0:128, cols 128:256 = channels 128:160
    # zero-padded (so the second relu writes exact zeros in rows 32:128)
    qw1t16 = qw1.transpose(2, 1, 0).reshape(240, 160) * 16.0
    qw1sw = np.zeros((256, 256), np.float32)
    qw1sw[0:240, 0:128] = qw1t16[:, 0:128]
    qw1sw[0:240, 128:160] = qw1t16[:, 128:160]
    qw1s = np.ascontiguousarray(qw1sw.reshape(2, 128, 256).transpose(1, 0, 2)
                                ).astype(e4).view(np.uint8)

    # stacked q conv2 weights (unscaled fp8): [k, r, o] = qw2[o, 128r+k]
    qw2sw = np.zeros((256, N_ATT), np.float32)
    qw2sw[0:160] = qw2[:, :, 0].T
    qw2s = np.ascontiguousarray(qw2sw.reshape(2, 128, N_ATT).transpose(1, 0, 2)
                                ).astype(e4).view(np.uint8)

    # packed q weights [128, 720] bf16
    qwp = np.zeros((128, 720), np.float32)
    qw1t = qw1.transpose(2, 1, 0)                       # [3, 80, 160]
    for d in range(3):
        qwp[0:N_MEL, 160 * d:160 * (d + 1)] = qw1t[d]
    qw2t = qw2[:, :, 0].T                               # [160, 80]
    qwp[0:128, 480:560] = qw2t[0:128]
    qwp[0:32, 560:640] = qw2t[128:160]
    qwp[0:N_MEL, 640:720] = qw3[:, :, 0].T
    qwp = qwp.astype(bf16)

    # packed biases [128, 13] f32
    qbp = np.zeros((128, 13), np.float32)
    qbp[0:128, 0] = qb1[0:128]
    qbp[0:32, 1] = qb1[128:160]
    qbp[0:N_MEL, 2] = qb2
    qbp[0:N_ATT, 3] = qb3
    qbp[0:N_ATT, 4] = kb2 * FOLD
    kb1_2d = kb1.reshape(8, 128)
    for j in range(8):
        qbp[0:128, 5 + j] = kb1_2d[j] * WS1

    # k path (fp8): keys+style^T, padded: [B, k(128), p(2), r(2), 402]
    ksum = np.zeros((B, N_TEXT, T2 + 2), np.float32)
    ksum[:, :, 1:T2 + 1] = keys + style.transpose(0, 2, 1)
    ks8 = ksum.reshape(B, 2, 2, 128, T2 + 2).transpose(0, 3, 1, 2, 4)
    ks8 = np.ascontiguousarray(ks8).astype(e4).view(np.uint8)

    # kw1 x8: [p, k(128), d(3), j(8), r(2), m(128)]
    kw1t = kw1.transpose(2, 1, 0) * WS1                 # [3, 512, 1024]
    kw1f = kw1t.reshape(3, 2, 2, 128, 8, 128)           # [d,p,r,k,j,m]
    kw1f = kw1f.transpose(1, 3, 0, 4, 2, 5)              # [p,k,d,j,r,m]
    kw1f = np.ascontiguousarray(kw1f).astype(e4).view(np.uint8)

    # kw2 x16: [k(128), p(4), r(2), o(80)]
    kw2t = kw2[:, :, 0].T * WS2                          # [1024, 80]
    kw2f = kw2t.reshape(4, 2, 128, N_ATT).transpose(2, 0, 1, 3)
    kw2f = np.ascontiguousarray(kw2f).astype(e4).view(np.uint8)

    shared = dict(qwp=qwp, qbp=qbp, kw1f=kw1f, kw2f=kw2f, qw1s=qw1s,
                  qw2s=qw2s)
    in_maps = []
    for c in range(N_CORES):
        sl = slice(c * BPC, (c + 1) * BPC)
        m = dict(shared)
        m["q8s"] = np.ascontiguousarray(q8s[sl])
        m["ks8"] = np.ascontiguousarray(ks8[sl])
        in_maps.append(m)
    return in_maps


def unpack_out(res_list, name="out_e"):
    """[BPC, NGRP, T1C, GRP, T2] f16 per core -> e [nb, 1, T1, T2] f32."""
    o = np.concatenate([np.asarray(r[name]) for r in res_list], axis=0)
    nb = o.shape[0]
    o = o.astype(np.float32).transpose(0, 1, 3, 2, 4).reshape(nb, 1, T1, T2)
    return np.ascontiguousarray(o)


def finish_host(e, attn_prior, mask):
    """From e = exp(z): logp = ln(e*(prior+eps)) - ln(rsum);
    attn = e*(prior+eps)*keep, renormalized."""
    nb = e.shape[0]
    prior = np.asarray(attn_prior, np.float32)[:nb]
    rsum = e.sum(axis=3, keepdims=True)                 # [nb,1,T1,1]
    ep = e * (prior[:, None] + EPS)
    logp = np.log(ep) - np.log(rsum)
    keep = ~np.asarray(mask[:nb, :, 0], bool)           # [nb, T2]
    u = ep * keep[:, None, None, :]
    u /= u.sum(axis=3, keepdims=True)
    return u.astype(np.float32), logp.astype(np.float32)


def run_cores(inputs, trace=False):
    from concourse.bass_utils import run_bass_kernel_spmd

    nc = get_program()
    in_maps = make_in_maps(inputs)
    res = run_bass_kernel_spmd(nc, in_maps, list(range(N_CORES)), trace=trace)
    e = unpack_out(res.results)
    attn, logp = finish_host(e, inputs["attn_prior"], np.asarray(inputs["mask"]))
    return attn, logp, res


def kernel(**inputs):
    attn, logp, _ = run_cores(inputs, trace=False)
    return attn, logp
